# revision 45
# baseline (speedup 1.0000x reference)
"""Multi-head causal attention (B=4, S=2048, D=1024, H=16, HD=64) on 8 trn2 cores.

Sharding: batch x head-group. Core c handles batch b = c//2 and heads
g*8..(g+1)*8 where g = c%2 (512 projection dims). The host sums the two
partial output projections per batch and adds the bias.

All matmuls run in bf16 (fp32r lowers to fp32_mode=HIGH on HW at ~2.5
cycles/row; bf16 streams at 1 cycle/row), accumulation stays fp32 in PSUM.

Schedule: the attention inner loops are paced by the ACT engine (only engine
able to exp). To keep the tensor engine from idling there, pure-tensor work
is interleaved via a filler queue: projections of the NEXT token chunk, V
transposes, and the output projection of the PREVIOUS chunk are emitted one
item per attention iteration. The P@V matmul is software-pipelined one
iteration behind its exp.

Per-core layout:
  - x[b] fed pre-transposed as [128, 8, 2048] bf16; QT/KT/VT [128, 4, 2048].
  - V PE-transposed into natural [kv, hp, hh, 65] with a fused ones column
    (P@V computes ctx and the softmax denominator together).
  - scores transposed (S_T[kv, q] = KT.T @ QT) per 128-kv-tile x 512-q-chunk;
    both heads of a pair share one 2-bank PSUM region so one ACT exp covers
    them. Causal: above-diagonal tiles skipped, diagonal blocks masked with
    bf16 multiplies on DVE.
  - normalization: DVE reciprocal of the denominator row, broadcast across
    partitions with a K=1 ones matmul; the odd head's normalized ctx is
    DMA-moved to partitions 64..127 so the output projection runs as a
    single K=128 accumulation chain per tile.
"""

from collections import deque
from contextlib import ExitStack

import numpy as np

import concourse.bass as bass
import concourse.tile as tile
from concourse import bacc, mybir
from concourse.bass_utils import run_bass_kernel_spmd

F32 = mybir.dt.float32
BF16 = mybir.dt.bfloat16
AF = mybir.ActivationFunctionType

B, S, D, H = 4, 2048, 1024, 16
HD = D // H          # 64
SCALE = float(np.sqrt(HD))
NCORES = 8
G = 2                # head groups (cores per batch)
HPC = H // G         # heads per core = 8
CW = HPC * HD        # per-core projection width = 512
KO = D // 128        # 8 contraction subtiles
OT = CW // 128       # 4 projection out-tiles (head pairs)
QCH = 512            # q chunk
NQT = S // 128       # 16 kv tiles
NCH = S // QCH       # 4 q chunks


def _emit(nc):
    XTB = nc.dram_tensor("XTB", [128, KO, S], BF16, kind="ExternalInput").ap()
    WQT = nc.dram_tensor("WQT", [128, KO, OT, 128], BF16, kind="ExternalInput").ap()
    WKT = nc.dram_tensor("WKT", [128, KO, OT, 128], BF16, kind="ExternalInput").ap()
    WVT = nc.dram_tensor("WVT", [128, KO, OT, 128], BF16, kind="ExternalInput").ap()
    WOT = nc.dram_tensor("WOT", [128, OT, D], BF16, kind="ExternalInput").ap()
    CMASK = nc.dram_tensor("CMASK", [128, 128], BF16, kind="ExternalInput").ap()
    ONESB = nc.dram_tensor("ONESB", [128, HD], BF16, kind="ExternalInput").ap()
    OUT = nc.dram_tensor("OUT", [S, D], BF16, kind="ExternalOutput").ap()

    with tile.TileContext(nc) as tc, ExitStack() as ctx, \
            nc.allow_low_precision(reason="bf16 attention pipeline"):
        consts = ctx.enter_context(tc.tile_pool(name="consts", bufs=1))
        xpool = ctx.enter_context(tc.tile_pool(name="xpool", bufs=2))
        qkv = ctx.enter_context(tc.tile_pool(name="qkv", bufs=1))
        ptp = ctx.enter_context(tc.tile_pool(name="ptp", bufs=8))
        rpool = ctx.enter_context(tc.tile_pool(name="rpool", bufs=4))
        cpool = ctx.enter_context(tc.tile_pool(name="cpool", bufs=2))
        cnp = ctx.enter_context(tc.tile_pool(name="cnp", bufs=2))
        opool = ctx.enter_context(tc.tile_pool(name="opool", bufs=4))
        ps_st = ctx.enter_context(tc.tile_pool(name="ps_st", bufs=2, space="PSUM"))
        ps_cx = ctx.enter_context(tc.tile_pool(name="ps_cx", bufs=2, space="PSUM"))
        ps_mm = ctx.enter_context(tc.tile_pool(name="ps_mm", bufs=2, space="PSUM"))

        wq = consts.tile([128, KO, OT, 128], BF16, tag="wq")
        wk = consts.tile([128, KO, OT, 128], BF16, tag="wk")
        wv = consts.tile([128, KO, OT, 128], BF16, tag="wv")
        wo = consts.tile([128, OT, D], BF16, tag="wo")
        cmask = consts.tile([128, 128], BF16, tag="cmask")
        onesb = consts.tile([128, HD], BF16, tag="onesb")
        # weights ride the Activation hwdge queue so the first projection
        # matmul only waits for its own weight, not the x-chunk stream.
        nc.scalar.dma_start(wv[:], WVT[:])
        nc.scalar.dma_start(wq[:], WQT[:])
        nc.scalar.dma_start(wk[:], WKT[:])
        nc.scalar.dma_start(cmask[:], CMASK[:])
        nc.scalar.dma_start(onesb[:], ONESB[:])
        nc.scalar.dma_start(wo[:], WOT[:])

        qt = qkv.tile([128, OT, S], BF16, tag="qt")
        kt = qkv.tile([128, OT, S], BF16, tag="kt")
        # v natural: [kv, kvtile, hp, hh, 65] with a ones column at 64.
        v_sb = qkv.tile([128, NQT, OT, 2, HD + 1], BF16, tag="v")
        nc.vector.memset(v_sb[:, :, :, :, HD:HD + 1], 1.0)

        state = {}
        fillers = deque()

        def emit_xdma(c):
            xt = xpool.tile([128, KO, QCH], BF16, tag="xt", name=f"xt{c}")
            for k in range(KO):  # per-ktile pieces so proj can start early
                nc.sync.dma_start(xt[:, k, :], XTB[:, k, bass.ts(c, QCH)])
            state["xt", c] = xt

        def proj_item(c, w, dst, ot):
            def f():
                xt = state["xt", c]
                pp = ps_st.tile([128, QCH], F32, tag="st", name="pp")
                for k in range(KO):
                    nc.tensor.matmul(pp[:], w[:, k, ot, :], xt[:, k, :],
                                     start=(k == 0), stop=(k == KO - 1))
                nc.vector.tensor_copy(dst[:, ot, bass.ts(c, QCH)], pp[:])
            return f

        def vdir_item(c, i):
            """V in natural [kv, outdim] layout directly: x-tile stationary,
            Wv moving - no PE transpose needed."""
            def f():
                xt = state["xt", c]
                pp = ps_st.tile([128, QCH], F32, tag="st", name="vd")
                for k in range(KO):
                    nc.tensor.matmul(
                        pp[:], xt[:, k, (i % 4) * 128:(i % 4 + 1) * 128],
                        wv[:, k, :, :].rearrange("p o n -> p (o n)"),
                        start=(k == 0), stop=(k == KO - 1))
                nc.vector.tensor_copy(
                    v_sb[:, i, :, :, 0:HD],
                    pp[:].rearrange("p (o h d) -> p o h d", o=OT, h=2))
            return f

        def po_item(j, t):
            def f():
                cn = state["cn", j]
                ob = opool.tile([128, 2, QCH], BF16, tag="ob")
                for dc in range(2):
                    po = ps_mm.tile([128, QCH], F32, tag="mm", name="po")
                    for k in range(OT):
                        nc.tensor.matmul(po[:], cn[:, k, bass.ts(t, 128)],
                                         wo[:, k, bass.ts(dc, QCH)],
                                         start=(k == 0), stop=(k == OT - 1))
                    nc.vector.tensor_copy(ob[:, dc, :], po[:])
                # one full-row write per q-tile on the Activation hwdge
                # queue so the latency-critical small norm DMAs on sync
                # aren't stuck behind it.
                nc.scalar.dma_start(
                    OUT[(4 * j + t) * 128:(4 * j + t + 1) * 128, :],
                    ob[:].rearrange("p a b -> p (a b)"))
            return f

        def push_chunk_fillers(c):
            """proj+vtrans for chunk c (to drain during chunk c-1). V first
            so its transposes overlap the q/k projections in the prologue."""
            fillers.append(vdir_item(c, 4 * c))
            fillers.append(vdir_item(c, 4 * c + 1))
            fillers.append(proj_item(c, wq, qt, 0))
            fillers.append(proj_item(c, wk, kt, 0))
            fillers.append(vdir_item(c, 4 * c + 2))
            fillers.append(vdir_item(c, 4 * c + 3))
            for ot in range(1, OT):
                fillers.append(proj_item(c, wq, qt, ot))
                fillers.append(proj_item(c, wk, kt, ot))

        def norm_a(hp, cx0, cx1):
            """Copy ctx+denom to SBUF (frees the cx PSUM banks), compute
            1/d partition-parallel: the two [1,512] denominator rows are
            DMA-reshaped to [128,8], one cheap DVE reciprocal covers both
            heads, and the result is DMA-reshaped back to rows."""
            ub0 = cpool.tile([HD + 1, QCH], F32, tag="ub0")
            ub1 = cpool.tile([HD + 1, QCH], F32, tag="ub1")
            nc.vector.tensor_copy(ub0[:], cx0[0:HD + 1, :])
            nc.vector.tensor_copy(ub1[:], cx1[0:HD + 1, :])
            dsq = rpool.tile([128, 8], F32, tag="dsq")
            nc.sync.dma_start(dsq[:, 0:4], ub0[HD:HD + 1, :])
            nc.sync.dma_start(dsq[:, 4:8], ub1[HD:HD + 1, :])
            dsr = rpool.tile([128, 8], BF16, tag="dsr")
            nc.vector.reciprocal(dsr[:], dsq[:])
            rcp0 = rpool.tile([HD + 1, QCH], BF16, tag="rcp0")
            rcp1 = rpool.tile([HD + 1, QCH], BF16, tag="rcp1")
            nc.sync.dma_start(rcp0[HD:HD + 1, :], dsr[:, 0:4])
            nc.sync.dma_start(rcp1[HD:HD + 1, :], dsr[:, 4:8])
            return ub0, ub1, rcp0, rcp1

        def norm_b(hp, cn, ub0, ub1, rcp0, rcp1):
            def f():
                rbt = ps_mm.tile([128, QCH], F32, tag="mm", name="rbt")
                nc.tensor.matmul(rbt[0:HD, :], onesb[HD:HD + 1, :],
                                 rcp0[HD:HD + 1, :], start=True, stop=True)
                rbt1 = ps_mm.tile([128, QCH], F32, tag="mm", name="rbt1")
                nc.tensor.matmul(rbt1[0:HD, :], onesb[HD:HD + 1, :],
                                 rcp1[HD:HD + 1, :], start=True, stop=True)
                nc.vector.tensor_mul(cn[0:HD, hp, :], ub0[0:HD, :],
                                     rbt[0:HD, :])
                ct1 = cpool.tile([HD, QCH], BF16, tag="ct1")
                nc.vector.tensor_mul(ct1[:], ub1[0:HD, :], rbt1[0:HD, :])
                nc.sync.dma_start(cn[HD:128, hp, :], ct1[:])
            return f

        def attention_chunk(j, deferred):
            last = 4 * j + 3
            cn = cnp.tile([128, OT, QCH], BF16, tag="cn", name=f"cn{j}")
            state["cn", j] = cn
            for hp in range(OT):
                cx0 = ps_cx.tile([128, QCH], F32, tag="cx", name="cx0")
                cx1 = ps_cx.tile([128, QCH], F32, tag="cx", name="cx1")
                pending = deque()  # (pt, s, i) awaiting their P@V
                for i in range(last + 1):
                    s = 128 * (i - 4 * j) if i >= 4 * j else 0
                    st = ps_st.tile([128, 2, QCH], F32, tag="st", name="st")
                    for hh in range(2):
                        hs = slice(HD * hh, HD * (hh + 1))
                        nc.tensor.matmul(
                            st[:, hh, s:QCH],
                            kt[hs, hp, bass.ts(i, 128)],
                            qt[hs, hp, j * QCH + s:(j + 1) * QCH],
                            start=True, stop=True)
                    pt = ptp.tile([128, 2, QCH], BF16, tag="pt")
                    nc.scalar.activation(pt[:, :, s:QCH], st[:, :, s:QCH], AF.Exp)
                    if i >= 4 * j:
                        for hh in range(2):
                            nc.vector.tensor_mul(
                                pt[:, hh, s:s + 128], pt[:, hh, s:s + 128],
                                cmask[:])
                    if len(pending) >= 2:
                        ppt, ps_, pi = pending.popleft()
                        for hh, cx in ((0, cx0), (1, cx1)):
                            nc.tensor.matmul(cx[0:HD + 1, ps_:QCH],
                                             v_sb[:, pi, hp, hh, :],
                                             ppt[:, hh, ps_:QCH],
                                             start=(pi == 0), stop=False)
                    pending.append((pt, s, i))
                    if i == 3 and deferred:
                        deferred.pop(0)()  # prev head-pair's normalize
                    if fillers:
                        fillers.popleft()()
                while pending:
                    ppt, ps_, pi = pending.popleft()
                    for hh, cx in ((0, cx0), (1, cx1)):
                        nc.tensor.matmul(cx[0:HD + 1, ps_:QCH],
                                         v_sb[:, pi, hp, hh, :],
                                         ppt[:, hh, ps_:QCH],
                                         start=(pi == 0), stop=(pi == last))
                deferred.append(norm_b(hp, cn, *norm_a(hp, cx0, cx1)))
            while fillers:
                fillers.popleft()()
            return deferred

        # ---- main schedule ----
        emit_xdma(0)
        push_chunk_fillers(0)
        while fillers:               # chunk 0 prologue runs serially
            fillers.popleft()()
        deferred = []
        for c in range(NCH):
            if c + 1 < NCH:
                emit_xdma(c + 1)
                push_chunk_fillers(c + 1)
            if c == NCH - 1:
                # no next-chunk proj fillers ahead of the po items here, so
                # the previous chunk's trailing normalize must flush first.
                for f in deferred:
                    f()
                deferred = []
            if c > 0:
                for t in range(4):
                    fillers.append(po_item(c - 1, t))
            deferred = attention_chunk(c, deferred)
        # final chunk epilogue: overlap the last normalize with partial
        # output-projection chains that don't need the last head-pair.
        j = NCH - 1
        cn = state["cn", j]
        partials = []
        for t, dc in ((0, 0), (0, 1)):
            po = ps_cx.tile([128, QCH], F32, tag="cx", name="pox")
            for k in range(OT - 1):
                nc.tensor.matmul(po[:], cn[:, k, bass.ts(t, 128)],
                                 wo[:, k, bass.ts(dc, QCH)],
                                 start=(k == 0), stop=False)
            partials.append((po, t, dc))
        for f in deferred:
            f()
        obx = opool.tile([128, 2, QCH], BF16, tag="ob")
        for po, t, dc in partials:
            nc.tensor.matmul(po[:], cn[:, OT - 1, bass.ts(t, 128)],
                             wo[:, OT - 1, bass.ts(dc, QCH)],
                             start=False, stop=True)
            nc.vector.tensor_copy(obx[:, dc, :], po[:])
        nc.scalar.dma_start(OUT[0 + (4 * j) * 128:(4 * j + 1) * 128, :],
                            obx[:].rearrange("p a b -> p (a b)"))
        for t in range(1, 4):
            po_item(j, t)()


_CACHE = {}


def _build():
    nc = bacc.Bacc("TRN2", target_bir_lowering=False, debug=False,
                   num_devices=NCORES)
    _emit(nc)
    nc.compile()
    return nc


def _in_maps(x, Wq, Wk, Wv, Wo):
    import ml_dtypes
    bf16 = ml_dtypes.bfloat16
    x = np.asarray(x, dtype=np.float32)
    Wq = np.asarray(Wq, dtype=np.float32)
    Wk = np.asarray(Wk, dtype=np.float32)
    Wv = np.asarray(Wv, dtype=np.float32)
    Wo = np.asarray(Wo, dtype=np.float32)

    cmask = np.triu(np.ones((128, 128), np.float32)).astype(bf16)
    onesb = np.ones((128, HD), np.float32).astype(bf16)

    # x[b] -> [128, KO, S]: [p, k, s] = x[b, s, k*128+p]
    xtb = [np.ascontiguousarray(
        x[b].T.reshape(KO, 128, S).transpose(1, 0, 2)).astype(bf16)
        for b in range(B)]

    def wslice(W, g, scale=1.0):
        # [p, k, ot, n] = W_g[ot*128+n, k*128+p]
        wg = (W[g * CW:(g + 1) * CW, :] * scale).astype(np.float32)
        return np.ascontiguousarray(
            wg.reshape(OT, 128, KO, 128).transpose(3, 2, 0, 1)).astype(bf16)

    def woslice(Wo, g):
        # [p, kt, d] = Wo[d, g*512 + kt*128 + p]
        wg = Wo[:, g * CW:(g + 1) * CW].astype(np.float32)
        return np.ascontiguousarray(
            wg.reshape(D, OT, 128).transpose(2, 1, 0)).astype(bf16)

    wmaps = []
    for g in range(G):
        wmaps.append({
            "WQT": wslice(Wq, g, scale=1.0 / SCALE),
            "WKT": wslice(Wk, g),
            "WVT": wslice(Wv, g),
            "WOT": woslice(Wo, g),
        })

    maps = []
    for c in range(NCORES):
        b, g = c // G, c % G
        m = {"XTB": xtb[b], "CMASK": cmask, "ONESB": onesb}
        m.update(wmaps[g])
        maps.append(m)
    return maps


def _run(x, Wq, Wk, Wv, Wo, bo, trace=False):
    nc = _CACHE.get("nc")
    if nc is None:
        nc = _CACHE["nc"] = _build()
    maps = _in_maps(x, Wq, Wk, Wv, Wo)
    res = run_bass_kernel_spmd(nc, maps, list(range(NCORES)), trace=trace)
    bo = np.asarray(bo, dtype=np.float32)
    out = np.empty((B, S, D), dtype=np.float32)
    for b in range(B):
        out[b] = (res.results[G * b]["OUT"].astype(np.float32)
                  + res.results[G * b + 1]["OUT"].astype(np.float32) + bo)
    return out, res


def kernel(x, Wq, Wk, Wv, Wo, bo):
    out, _ = _run(x, Wq, Wk, Wv, Wo, bo)
    return out


# revision 46
# speedup vs baseline: 1.1905x; 1.1905x over previous
"""Multi-head causal attention (B=4, S=2048, D=1024, H=16, HD=64) on 8 trn2 cores.

Sharding: batch x head-group. Core c handles batch b = c//2 and heads
g*8..(g+1)*8 where g = c%2 (512 projection dims). The host sums the two
partial output projections per batch and adds the bias.

All matmuls run in bf16 (fp32r lowers to fp32_mode=HIGH on HW at ~2.5
cycles/row; bf16 streams at 1 cycle/row), accumulation stays fp32 in PSUM.

Schedule: the attention inner loops are paced by the ACT engine (only engine
able to exp). To keep the tensor engine from idling there, pure-tensor work
is interleaved via a filler queue: projections of the NEXT token chunk, V
transposes, and the output projection of the PREVIOUS chunk are emitted one
item per attention iteration. The P@V matmul is software-pipelined one
iteration behind its exp.

Per-core layout:
  - x[b] fed pre-transposed as [128, 8, 2048] bf16; QT/KT/VT [128, 4, 2048].
  - V PE-transposed into natural [kv, hp, hh, 65] with a fused ones column
    (P@V computes ctx and the softmax denominator together).
  - scores transposed (S_T[kv, q] = KT.T @ QT) per 128-kv-tile x 512-q-chunk;
    both heads of a pair share one 2-bank PSUM region so one ACT exp covers
    them. Causal: above-diagonal tiles skipped, diagonal blocks masked with
    bf16 multiplies on DVE.
  - normalization: DVE reciprocal of the denominator row, broadcast across
    partitions with a K=1 ones matmul; the odd head's normalized ctx is
    DMA-moved to partitions 64..127 so the output projection runs as a
    single K=128 accumulation chain per tile.
"""

from collections import deque
from contextlib import ExitStack

import numpy as np

import concourse.bass as bass
import concourse.tile as tile
from concourse import bacc, mybir
from concourse.bass_utils import run_bass_kernel_spmd

F32 = mybir.dt.float32
BF16 = mybir.dt.bfloat16
AF = mybir.ActivationFunctionType

B, S, D, H = 4, 2048, 1024, 16
HD = D // H          # 64
SCALE = float(np.sqrt(HD))
NCORES = 8
G = 2                # head groups (cores per batch)
HPC = H // G         # heads per core = 8
CW = HPC * HD        # per-core projection width = 512
KO = D // 128        # 8 contraction subtiles
OT = CW // 128       # 4 projection out-tiles (head pairs)
QCH = 512            # q chunk
NQT = S // 128       # 16 kv tiles
NCH = S // QCH       # 4 q chunks


def _emit(nc):
    XTB = nc.dram_tensor("XTB", [128, KO, S], BF16, kind="ExternalInput").ap()
    WQT = nc.dram_tensor("WQT", [128, KO, OT, 128], BF16, kind="ExternalInput").ap()
    WKT = nc.dram_tensor("WKT", [128, KO, OT, 128], BF16, kind="ExternalInput").ap()
    WVT = nc.dram_tensor("WVT", [128, KO, OT, 128], BF16, kind="ExternalInput").ap()
    WOT = nc.dram_tensor("WOT", [128, OT, D], BF16, kind="ExternalInput").ap()
    CMASK = nc.dram_tensor("CMASK", [128, 128], BF16, kind="ExternalInput").ap()
    ONESB = nc.dram_tensor("ONESB", [128, HD], BF16, kind="ExternalInput").ap()
    OUT = nc.dram_tensor("OUT", [S, D], BF16, kind="ExternalOutput").ap()

    with tile.TileContext(nc) as tc, ExitStack() as ctx, \
            nc.allow_low_precision(reason="bf16 attention pipeline"):
        consts = ctx.enter_context(tc.tile_pool(name="consts", bufs=1))
        xpool = ctx.enter_context(tc.tile_pool(name="xpool", bufs=2))
        qkv = ctx.enter_context(tc.tile_pool(name="qkv", bufs=1))
        ptp = ctx.enter_context(tc.tile_pool(name="ptp", bufs=8))
        rpool = ctx.enter_context(tc.tile_pool(name="rpool", bufs=4))
        cpool = ctx.enter_context(tc.tile_pool(name="cpool", bufs=2))
        cnp = ctx.enter_context(tc.tile_pool(name="cnp", bufs=2))
        opool = ctx.enter_context(tc.tile_pool(name="opool", bufs=4))
        ps_st = ctx.enter_context(tc.tile_pool(name="ps_st", bufs=2, space="PSUM"))
        ps_cx = ctx.enter_context(tc.tile_pool(name="ps_cx", bufs=2, space="PSUM"))
        ps_mm = ctx.enter_context(tc.tile_pool(name="ps_mm", bufs=2, space="PSUM"))

        wq = consts.tile([128, KO, OT, 128], BF16, tag="wq")
        wk = consts.tile([128, KO, OT, 128], BF16, tag="wk")
        wv = consts.tile([128, KO, OT, 128], BF16, tag="wv")
        wo = consts.tile([128, OT, D], BF16, tag="wo")
        cmask = consts.tile([128, 128], BF16, tag="cmask")
        onesb = consts.tile([128, HD], BF16, tag="onesb")
        # weights ride the Activation hwdge queue so the first projection
        # matmul only waits for its own weight, not the x-chunk stream.
        nc.scalar.dma_start(wv[:], WVT[:])
        nc.scalar.dma_start(wq[:], WQT[:])
        nc.scalar.dma_start(wk[:], WKT[:])
        nc.scalar.dma_start(cmask[:], CMASK[:])
        nc.scalar.dma_start(onesb[:], ONESB[:])
        nc.scalar.dma_start(wo[:], WOT[:])

        qt = qkv.tile([128, OT, S], BF16, tag="qt")
        kt = qkv.tile([128, OT, S], BF16, tag="kt")
        # v natural: [kv, kvtile, hp, hh, 65] with a ones column at 64.
        v_sb = qkv.tile([128, NQT, OT, 2, HD + 1], BF16, tag="v")
        nc.vector.memset(v_sb[:, :, :, :, HD:HD + 1], 1.0)

        state = {}
        fillers = deque()

        def emit_xdma(c):
            xt = xpool.tile([128, KO, QCH], BF16, tag="xt", name=f"xt{c}")
            for k in range(KO):  # per-ktile pieces so proj can start early
                nc.sync.dma_start(xt[:, k, :], XTB[:, k, bass.ts(c, QCH)])
            state["xt", c] = xt

        def proj_item(c, w, dst, ot):
            def f():
                xt = state["xt", c]
                pp = ps_st.tile([128, QCH], F32, tag="st", name="pp")
                for k in range(KO):
                    nc.tensor.matmul(pp[:], w[:, k, ot, :], xt[:, k, :],
                                     start=(k == 0), stop=(k == KO - 1))
                nc.vector.tensor_copy(dst[:, ot, bass.ts(c, QCH)], pp[:])
            return f

        def vdir_item(c, i):
            """V in natural [kv, outdim] layout directly: x-tile stationary,
            Wv moving - no PE transpose needed."""
            def f():
                xt = state["xt", c]
                pp = ps_st.tile([128, QCH], F32, tag="st", name="vd")
                for k in range(KO):
                    nc.tensor.matmul(
                        pp[:], xt[:, k, (i % 4) * 128:(i % 4 + 1) * 128],
                        wv[:, k, :, :].rearrange("p o n -> p (o n)"),
                        start=(k == 0), stop=(k == KO - 1))
                nc.vector.tensor_copy(
                    v_sb[:, i, :, :, 0:HD],
                    pp[:].rearrange("p (o h d) -> p o h d", o=OT, h=2))
            return f

        def po_item(j, t, dc):
            def f():
                cn = state["cn", j]
                po = ps_mm.tile([128, QCH], F32, tag="mm", name="po")
                for k in range(OT):
                    nc.tensor.matmul(po[:], cn[:, k, bass.ts(t, 128)],
                                     wo[:, k, bass.ts(dc, QCH)],
                                     start=(k == 0), stop=(k == OT - 1))
                ob = opool.tile([128, QCH], BF16, tag="ob")
                nc.vector.tensor_copy(ob[:], po[:])
                # output writes ride the Activation hwdge queue so the
                # latency-critical small norm DMAs on sync aren't stuck
                # behind them.
                nc.scalar.dma_start(
                    OUT[(4 * j + t) * 128:(4 * j + t + 1) * 128,
                        bass.ts(dc, QCH)],
                    ob[:])
            return f

        def push_chunk_fillers(c):
            """proj+vtrans for chunk c (to drain during chunk c-1). V first
            so its transposes overlap the q/k projections in the prologue."""
            fillers.append(vdir_item(c, 4 * c))
            fillers.append(vdir_item(c, 4 * c + 1))
            fillers.append(proj_item(c, wq, qt, 0))
            fillers.append(proj_item(c, wk, kt, 0))
            fillers.append(vdir_item(c, 4 * c + 2))
            fillers.append(vdir_item(c, 4 * c + 3))
            for ot in range(1, OT):
                fillers.append(proj_item(c, wq, qt, ot))
                fillers.append(proj_item(c, wk, kt, ot))

        def norm_a(hp, cx0, cx1):
            """Copy ctx+denom to SBUF (frees the cx PSUM banks), compute
            1/d partition-parallel: the two [1,512] denominator rows are
            DMA-reshaped to [128,8], one cheap DVE reciprocal covers both
            heads, and the result is DMA-reshaped back to rows."""
            ub0 = cpool.tile([HD + 1, QCH], F32, tag="ub0")
            ub1 = cpool.tile([HD + 1, QCH], F32, tag="ub1")
            nc.vector.tensor_copy(ub0[:], cx0[0:HD + 1, :])
            nc.vector.tensor_copy(ub1[:], cx1[0:HD + 1, :])
            dsq = rpool.tile([128, 8], F32, tag="dsq")
            nc.sync.dma_start(dsq[:, 0:4], ub0[HD:HD + 1, :])
            nc.sync.dma_start(dsq[:, 4:8], ub1[HD:HD + 1, :])
            dsr = rpool.tile([128, 8], BF16, tag="dsr")
            nc.vector.reciprocal(dsr[:], dsq[:])
            rcp0 = rpool.tile([HD + 1, QCH], BF16, tag="rcp0")
            rcp1 = rpool.tile([HD + 1, QCH], BF16, tag="rcp1")
            nc.sync.dma_start(rcp0[HD:HD + 1, :], dsr[:, 0:4])
            nc.sync.dma_start(rcp1[HD:HD + 1, :], dsr[:, 4:8])
            return ub0, ub1, rcp0, rcp1

        def norm_b(hp, cn, ub0, ub1, rcp0, rcp1):
            def f():
                rbt = ps_mm.tile([128, QCH], F32, tag="mm", name="rbt")
                nc.tensor.matmul(rbt[0:HD, :], onesb[HD:HD + 1, :],
                                 rcp0[HD:HD + 1, :], start=True, stop=True)
                rbt1 = ps_mm.tile([128, QCH], F32, tag="mm", name="rbt1")
                nc.tensor.matmul(rbt1[0:HD, :], onesb[HD:HD + 1, :],
                                 rcp1[HD:HD + 1, :], start=True, stop=True)
                nc.vector.tensor_mul(cn[0:HD, hp, :], ub0[0:HD, :],
                                     rbt[0:HD, :])
                ct1 = cpool.tile([HD, QCH], BF16, tag="ct1")
                nc.vector.tensor_mul(ct1[:], ub1[0:HD, :], rbt1[0:HD, :])
                nc.sync.dma_start(cn[HD:128, hp, :], ct1[:])
            return f

        def attention_chunk(j, deferred):
            last = 4 * j + 3
            cn = cnp.tile([128, OT, QCH], BF16, tag="cn", name=f"cn{j}")
            state["cn", j] = cn
            for hp in range(OT):
                cx0 = ps_cx.tile([128, QCH], F32, tag="cx", name="cx0")
                cx1 = ps_cx.tile([128, QCH], F32, tag="cx", name="cx1")
                pending = deque()  # (pt, s, i) awaiting their P@V
                for i in range(last + 1):
                    s = 128 * (i - 4 * j) if i >= 4 * j else 0
                    st = ps_st.tile([128, 2, QCH], F32, tag="st", name="st")
                    for hh in range(2):
                        hs = slice(HD * hh, HD * (hh + 1))
                        nc.tensor.matmul(
                            st[:, hh, s:QCH],
                            kt[hs, hp, bass.ts(i, 128)],
                            qt[hs, hp, j * QCH + s:(j + 1) * QCH],
                            start=True, stop=True)
                    pt = ptp.tile([128, 2, QCH], BF16, tag="pt")
                    nc.scalar.activation(pt[:, :, s:QCH], st[:, :, s:QCH], AF.Exp)
                    if i >= 4 * j:
                        for hh in range(2):
                            nc.vector.tensor_mul(
                                pt[:, hh, s:s + 128], pt[:, hh, s:s + 128],
                                cmask[:])
                    if len(pending) >= 2:
                        ppt, ps_, pi = pending.popleft()
                        for hh, cx in ((0, cx0), (1, cx1)):
                            nc.tensor.matmul(cx[0:HD + 1, ps_:QCH],
                                             v_sb[:, pi, hp, hh, :],
                                             ppt[:, hh, ps_:QCH],
                                             start=(pi == 0), stop=False)
                    pending.append((pt, s, i))
                    if i == 3 and deferred:
                        deferred.pop(0)()  # prev head-pair's normalize
                    if fillers:
                        fillers.popleft()()
                while pending:
                    ppt, ps_, pi = pending.popleft()
                    for hh, cx in ((0, cx0), (1, cx1)):
                        nc.tensor.matmul(cx[0:HD + 1, ps_:QCH],
                                         v_sb[:, pi, hp, hh, :],
                                         ppt[:, hh, ps_:QCH],
                                         start=(pi == 0), stop=(pi == last))
                deferred.append(norm_b(hp, cn, *norm_a(hp, cx0, cx1)))
            while fillers:
                fillers.popleft()()
            return deferred

        # ---- main schedule ----
        emit_xdma(0)
        push_chunk_fillers(0)
        while fillers:               # chunk 0 prologue runs serially
            fillers.popleft()()
        deferred = []
        for c in range(NCH):
            if c + 1 < NCH:
                emit_xdma(c + 1)
                push_chunk_fillers(c + 1)
            if c == NCH - 1:
                # no next-chunk proj fillers ahead of the po items here, so
                # the previous chunk's trailing normalize must flush first.
                for f in deferred:
                    f()
                deferred = []
            if c > 0:
                for t in range(4):
                    for dc in range(2):
                        fillers.append(po_item(c - 1, t, dc))
            deferred = attention_chunk(c, deferred)
        # final chunk epilogue: overlap the last normalize with partial
        # output-projection chains that don't need the last head-pair.
        j = NCH - 1
        cn = state["cn", j]
        partials = []
        for t, dc in ((0, 0), (0, 1)):
            po = ps_cx.tile([128, QCH], F32, tag="cx", name="pox")
            for k in range(OT - 1):
                nc.tensor.matmul(po[:], cn[:, k, bass.ts(t, 128)],
                                 wo[:, k, bass.ts(dc, QCH)],
                                 start=(k == 0), stop=False)
            partials.append((po, t, dc))
        for f in deferred:
            f()
        for po, t, dc in partials:
            nc.tensor.matmul(po[:], cn[:, OT - 1, bass.ts(t, 128)],
                             wo[:, OT - 1, bass.ts(dc, QCH)],
                             start=False, stop=True)
            ob = opool.tile([128, QCH], BF16, tag="ob")
            nc.vector.tensor_copy(ob[:], po[:])
            nc.scalar.dma_start(
                OUT[(4 * j + t) * 128:(4 * j + t + 1) * 128,
                    bass.ts(dc, QCH)],
                ob[:])
        for t in range(4):
            for dc in range(2):
                if t == 0:
                    continue
                po_item(j, t, dc)()


_CACHE = {}


def _build():
    nc = bacc.Bacc("TRN2", target_bir_lowering=False, debug=False,
                   num_devices=NCORES)
    _emit(nc)
    nc.compile()
    return nc


def _in_maps(x, Wq, Wk, Wv, Wo):
    import ml_dtypes
    bf16 = ml_dtypes.bfloat16
    x = np.asarray(x, dtype=np.float32)
    Wq = np.asarray(Wq, dtype=np.float32)
    Wk = np.asarray(Wk, dtype=np.float32)
    Wv = np.asarray(Wv, dtype=np.float32)
    Wo = np.asarray(Wo, dtype=np.float32)

    cmask = np.triu(np.ones((128, 128), np.float32)).astype(bf16)
    onesb = np.ones((128, HD), np.float32).astype(bf16)

    # x[b] -> [128, KO, S]: [p, k, s] = x[b, s, k*128+p]
    xtb = [np.ascontiguousarray(
        x[b].T.reshape(KO, 128, S).transpose(1, 0, 2)).astype(bf16)
        for b in range(B)]

    def wslice(W, g, scale=1.0):
        # [p, k, ot, n] = W_g[ot*128+n, k*128+p]
        wg = (W[g * CW:(g + 1) * CW, :] * scale).astype(np.float32)
        return np.ascontiguousarray(
            wg.reshape(OT, 128, KO, 128).transpose(3, 2, 0, 1)).astype(bf16)

    def woslice(Wo, g):
        # [p, kt, d] = Wo[d, g*512 + kt*128 + p]
        wg = Wo[:, g * CW:(g + 1) * CW].astype(np.float32)
        return np.ascontiguousarray(
            wg.reshape(D, OT, 128).transpose(2, 1, 0)).astype(bf16)

    wmaps = []
    for g in range(G):
        wmaps.append({
            "WQT": wslice(Wq, g, scale=1.0 / SCALE),
            "WKT": wslice(Wk, g),
            "WVT": wslice(Wv, g),
            "WOT": woslice(Wo, g),
        })

    maps = []
    for c in range(NCORES):
        b, g = c // G, c % G
        m = {"XTB": xtb[b], "CMASK": cmask, "ONESB": onesb}
        m.update(wmaps[g])
        maps.append(m)
    return maps


def _run(x, Wq, Wk, Wv, Wo, bo, trace=False):
    nc = _CACHE.get("nc")
    if nc is None:
        nc = _CACHE["nc"] = _build()
    maps = _in_maps(x, Wq, Wk, Wv, Wo)
    res = run_bass_kernel_spmd(nc, maps, list(range(NCORES)), trace=trace)
    bo = np.asarray(bo, dtype=np.float32)
    out = np.empty((B, S, D), dtype=np.float32)
    for b in range(B):
        out[b] = (res.results[G * b]["OUT"].astype(np.float32)
                  + res.results[G * b + 1]["OUT"].astype(np.float32) + bo)
    return out, res


def kernel(x, Wq, Wk, Wv, Wo, bo):
    out, _ = _run(x, Wq, Wk, Wv, Wo, bo)
    return out
